# revision 1
# baseline (speedup 1.0000x reference)
"""Trainium2 Bass kernel for a 4-layer LSTM autoencoder.

Contract: kernel(**inputs) takes the FULL fp32 inputs (B=65536) and returns
the full [B, T, D] fp32 reconstruction. Internally: pure data parallelism —
the batch is sharded across 8 NeuronCores; weights are replicated.

Device-side layout: everything is stored transposed, [feature=partitions,
batch=free]. Gates are computed as W_g @ x (+ W_hg @ h) with the batch
streaming through the PE array, so the recurrent state h never needs an
on-chip transpose. The host pre-transposes x and post-transposes the output.

The Activation engine is the throughput bottleneck (5 transcendental passes
per hidden element per layer-step at 1 elem/cycle/lane), so the output gate
is taken off ACT: sigma(o) is evaluated as a 5-segment piecewise-linear
min/max form (max error 1.74e-2 pointwise, ~4e-3 end-to-end) spread across
the engines:

    y  = M1*pre + (M1*b_o + 0.5)   PSUM -> SBUF bf16; ACT Identity (free
                                   affine) on t%3==2 steps, DVE
                                   tensor_scalar otherwise (load balance)
    l2 = A2*y + B2,  l2' = A2*y + B2'    (DVE tensor_scalar, 4x mode)
    t  = min(y, l2), u = max(t, l2')     (DVE tensor_tensor, 2x mode;
                                         walrus rejects TT min/max on Pool)
    o  = clamp(u, 0, 1)                  (GpSimd tensor_scalar)

Remaining ACT work per layer-step: sigma(i), tanh(g), sigma(f) as half-width
(N=1024) instructions off the double-buffered PSUM tiles, plus tanh(c)
halves off SBUF. The cell state c is kept in bf16 so the DVE elementwise
chain runs in 2x mode; t2 = f*c runs on GpSimd (gate order o,f,i,g starts
it early). Chain ops run at half granularity (clamp and h at quarters) so h
releases early for the next step's recurrent matmuls. Encoder layers (and
decoder layers) are pipelined with a 1-step skew so the recurrence latency
of one layer hides under the other layer's work; x-loads issue from the
GpSimd DMA queue so the next super-batch's encoder is not serialized behind
the previous decoder's output stores on the SP queue.
"""

import os
import sys
import time
from contextlib import ExitStack

import numpy as np

sys.path.insert(0, "/opt/trn_rl_repo")

import ml_dtypes  # noqa: E402

import concourse.bass as bass  # noqa: E402
import concourse.tile as tile  # noqa: E402
from concourse import bacc, mybir  # noqa: E402
from concourse.bass_utils import run_bass_kernel_spmd  # noqa: E402

F32 = mybir.dt.float32
BF16 = mybir.dt.bfloat16
SIG = mybir.ActivationFunctionType.Sigmoid
TANH = mybir.ActivationFunctionType.Tanh
IDENT = mybir.ActivationFunctionType.Identity
MULT = mybir.AluOpType.mult
ADD = mybir.AluOpType.add
MAXOP = mybir.AluOpType.max
MINOP = mybir.AluOpType.min

B, T, D, H, L = 65536, 8, 60, 128, 64
N_CORES = 8
B_CORE = B // N_CORES        # 8192
SBW = 2048                   # super-batch width (columns in flight)
N_SB = B_CORE // SBW         # 4
CHUNK = 512                  # matmul moving-operand width (one PSUM bank)
N_CHUNKS = SBW // CHUNK      # 4
HALF = SBW // 2              # 1024

# PWL sigmoid (minimax fit): sigma(x) ~= clip(max(min(M1 x+.5, M2 x+B2),
#                                               M2 x+(1-B2)), 0, 1)
PWL_M1 = 0.21579
PWL_M2 = 0.0600418
PWL_B2 = 0.75755993
# lines re-expressed in y = M1*x + 0.5:  l = A2*y + (B - 0.5*A2)
PWL_A2 = PWL_M2 / PWL_M1
PWL_BB2 = PWL_B2 - 0.5 * PWL_A2
PWL_BB2P = (1.0 - PWL_B2) - 0.5 * PWL_A2

# layer descriptors: (name, input feature dim incl. ones-row, bias-in-ACT?)
LAYERS = {
    "enc0": dict(kin=D + 1, act_bias=False),
    "enc1": dict(kin=H, act_bias=True),
    "dec0": dict(kin=L + 1, act_bias=False),
    "dec1": dict(kin=H, act_bias=True),
}
GATE_FUNCS = [SIG, SIG, TANH, SIG]  # PyTorch gate order: i, f, g, o
GATE_ORDER = [3, 1, 0, 2]           # emission order: o (PWL), f, i, g

_last_results = None  # set by kernel(); test harness reads exec_time_ns


def _build_kernel(trace: bool = False):
    nc = bacc.Bacc("TRN2", target_bir_lowering=False, debug=False,
                   num_devices=N_CORES)

    x_ext = nc.dram_tensor("x", [T, D + 1, B_CORE], BF16, kind="ExternalInput").ap()
    out_ext = nc.dram_tensor("out", [T, D, B_CORE], F32, kind="ExternalOutput").ap()

    w_in_ext, w_rec_ext, bias_ext, oline_ext = {}, {}, {}, {}
    for name, cfg in LAYERS.items():
        w_in_ext[name] = nc.dram_tensor(
            f"{name}_w_in", [cfg["kin"], 4 * H], BF16, kind="ExternalInput").ap()
        w_rec_ext[name] = nc.dram_tensor(
            f"{name}_w_rec", [H, 4 * H], BF16, kind="ExternalInput").ap()
        if cfg["act_bias"]:
            bias_ext[name] = nc.dram_tensor(
                f"{name}_bias", [H, 4], F32, kind="ExternalInput").ap()
        oline_ext[name] = nc.dram_tensor(
            f"{name}_oline", [H, 1], F32, kind="ExternalInput").ap()
    w_lat_ext = nc.dram_tensor("w_lat", [H, L], BF16, kind="ExternalInput").ap()
    b_lat_ext = nc.dram_tensor("b_lat", [L, 1], F32, kind="ExternalInput").ap()
    w_out_ext = nc.dram_tensor("w_out", [H, 64], BF16, kind="ExternalInput").ap()
    b_out_ext = nc.dram_tensor("b_out", [128, 1], F32, kind="ExternalInput").ap()

    with tile.TileContext(nc) as tc, ExitStack() as ctx:
        weights = ctx.enter_context(tc.tile_pool(name="weights", bufs=1))
        xpool = ctx.enter_context(tc.tile_pool(name="xpool", bufs=2))
        hpool = ctx.enter_context(tc.tile_pool(name="hpool", bufs=1))
        cpool = ctx.enter_context(tc.tile_pool(name="cpool", bufs=1))
        gpool = ctx.enter_context(tc.tile_pool(name="gpool", bufs=1))
        tpool = ctx.enter_context(tc.tile_pool(name="tpool", bufs=1))
        zpool = ctx.enter_context(tc.tile_pool(name="zpool", bufs=2))
        opool = ctx.enter_context(tc.tile_pool(name="opool", bufs=1))
        psA = ctx.enter_context(tc.tile_pool(name="psA", bufs=1, space="PSUM"))
        psB = ctx.enter_context(tc.tile_pool(name="psB", bufs=1, space="PSUM"))

        # ---- load weights once ----
        w_in, w_rec, w_bias, w_oline = {}, {}, {}, {}
        for name, cfg in LAYERS.items():
            w_in[name] = weights.tile([cfg["kin"], 4 * H], BF16, tag=f"wi_{name}", name=f"wi_{name}")
            nc.sync.dma_start(out=w_in[name], in_=w_in_ext[name][:, :])
            w_rec[name] = weights.tile([H, 4 * H], BF16, tag=f"wr_{name}", name=f"wr_{name}")
            nc.sync.dma_start(out=w_rec[name], in_=w_rec_ext[name][:, :])
            if cfg["act_bias"]:
                w_bias[name] = weights.tile([H, 4], F32, tag=f"wb_{name}", name=f"wb_{name}")
                nc.sync.dma_start(out=w_bias[name], in_=bias_ext[name][:, :])
            w_oline[name] = weights.tile([H, 1], F32, tag=f"wo_{name}", name=f"wo_{name}")
            nc.sync.dma_start(out=w_oline[name], in_=oline_ext[name][:, :])
        w_lat = weights.tile([H, L], BF16, tag="w_lat")
        nc.sync.dma_start(out=w_lat, in_=w_lat_ext[:, :])
        b_lat = weights.tile([L, 1], F32, tag="b_lat")
        nc.sync.dma_start(out=b_lat, in_=b_lat_ext[:, :])
        w_out = weights.tile([H, 64], BF16, tag="w_out")
        nc.sync.dma_start(out=w_out, in_=w_out_ext[:, :])
        b_out = weights.tile([128, 1], F32, tag="b_out")
        nc.sync.dma_start(out=b_out, in_=b_out_ext[:, :])

        def gate_half_mm(name, t, g, half, rhs_in, h_prev, ps_pool, ps_tag):
            """One gate-half [128, HALF] accumulated in a PSUM tile:
            input chunks first, then recurrent chunks (fewer weight swaps)."""
            kin = LAYERS[name]["kin"]
            gps = ps_pool.tile([H, HALF], F32, tag=ps_tag, bufs=2,
                               name=f"gps_{name}_{t}_{g}_{half}")
            for cc in range(N_CHUNKS // 2):
                c = half * (N_CHUNKS // 2) + cc
                nc.tensor.matmul(gps[:, bass.ts(cc, CHUNK)],
                                 w_in[name][:, bass.ts(g, H)],
                                 rhs_in[:kin, bass.ts(c, CHUNK)],
                                 start=True, stop=(t == 0))
            if t > 0:
                for cc in range(N_CHUNKS // 2):
                    c = half * (N_CHUNKS // 2) + cc
                    nc.tensor.matmul(gps[:, bass.ts(cc, CHUNK)],
                                     w_rec[name][:, bass.ts(g, H)],
                                     h_prev[:, bass.ts(c, CHUNK)],
                                     start=False, stop=True)
            return gps

        def lstm_step(name, t, rhs_in, h_prev, c_tile, ps_pool, ps_tag, cls,
                      hbufs):
            """Emit one LSTM step over SBW columns. Returns (h_new, c_tile)."""
            cfg = LAYERS[name]
            gates = [None] * 4
            y_t = None
            for g in GATE_ORDER:
                if t == 0 and g == 1:
                    continue  # forget gate unused when c == 0
                if g == 3:
                    # ---- output gate via PWL on DVE + GpSimd ----
                    y_t = tpool.tile([H, SBW], BF16, tag=f"y_{cls}", bufs=2,
                                     name=f"y_{name}_{t}")
                    s2 = w_oline[name][:, 0:1]
                    for half in range(2):
                        gps = gate_half_mm(name, t, g, half, rhs_in, h_prev,
                                           ps_pool, ps_tag)
                        if t % 3 == 2 or (t == 7 and half == 0):
                            # load balance: these slots drain the o-psum
                            # via ACT's free affine (Identity), rest via DVE;
                            # t=7 has no next-step recurrence pressure
                            nc.scalar.activation(
                                out=y_t[:, bass.ts(half, HALF)], in_=gps,
                                func=IDENT, bias=s2, scale=PWL_M1)
                        else:
                            nc.vector.tensor_scalar(
                                out=y_t[:, bass.ts(half, HALF)], in0=gps,
                                scalar1=PWL_M1, scalar2=s2, op0=MULT, op1=ADD)
                    l2 = tpool.tile([H, SBW], BF16, tag=f"l2_{cls}",
                                    name=f"l2_{name}_{t}")
                    l2p = tpool.tile([H, SBW], BF16, tag=f"l2p_{cls}",
                                     name=f"l2p_{name}_{t}")
                    tmin = tpool.tile([H, SBW], BF16, tag=f"tmin_{cls}", bufs=2,
                                      name=f"tmin_{name}_{t}")
                    umax = tpool.tile([H, SBW], BF16, tag=f"umax_{cls}",
                                      name=f"umax_{name}_{t}")
                    nc.vector.tensor_scalar(
                        out=l2, in0=y_t, scalar1=PWL_A2,
                        scalar2=PWL_BB2, op0=MULT, op1=ADD)
                    nc.vector.tensor_scalar(
                        out=l2p, in0=y_t, scalar1=PWL_A2,
                        scalar2=PWL_BB2P, op0=MULT, op1=ADD)
                    for half in range(2):
                        s = bass.ts(half, HALF)
                        nc.vector.tensor_tensor(tmin[:, s], y_t[:, s],
                                                l2[:, s], MINOP)
                        nc.vector.tensor_tensor(umax[:, s], tmin[:, s],
                                                l2p[:, s], MAXOP)
                    gates[3] = umax  # clamped later on DVE
                else:
                    gate = gpool.tile([H, SBW], BF16, tag=f"g{g}_{cls}",
                                      bufs=(2 if (g == 2 and cls == "A")
                                            else 1),
                                      name=f"gate{g}_{name}_{t}")
                    bias_arg = (w_bias[name][:, g:g + 1] if cfg["act_bias"]
                                else 0.0)
                    for half in range(2):
                        gps = gate_half_mm(name, t, g, half, rhs_in, h_prev,
                                           ps_pool, ps_tag)
                        nc.scalar.activation(
                            out=gate[:, bass.ts(half, HALF)], in_=gps,
                            func=GATE_FUNCS[g], bias=bias_arg)
                    gates[g] = gate
                    if g == 2 and t > 0:
                        t1 = tpool.tile([H, SBW], BF16, tag=f"t1_{cls}",
                                        name=f"t1_{name}_{t}")
                        for half in range(2):
                            s = bass.ts(half, HALF)
                            nc.vector.tensor_tensor(t1[:, s], gates[0][:, s],
                                                    gates[2][:, s], MULT)
                    if g == 1:
                        t2 = tpool.tile([H, SBW], BF16, tag=f"t2_{cls}",
                                        name=f"t2_{name}_{t}")
                        for half in range(2):
                            s = bass.ts(half, HALF)
                            nc.gpsimd.tensor_tensor(t2[:, s], gates[1][:, s],
                                                    c_tile[:, s], MULT)
            # ---- cell update (bf16 c; DVE in 2x mode), half granularity so
            # h releases early for the next step's recurrent matmuls ----
            if t == 0:
                c_tile = cpool.tile([H, SBW], BF16, tag=f"c_{name}",
                                    name=f"c_{name}_{t}")
            o_gate = gpool.tile([H, SBW], BF16, tag=f"g3_{cls}",
                                name=f"gate3_{name}_{t}")
            tc_t = tpool.tile([H, SBW], BF16, tag=f"tanhc_{cls}",
                              name=f"tanhc_{name}_{t}")
            h_new = hpool.tile([H, SBW], BF16, tag=f"h_{name}", bufs=hbufs,
                               name=f"h_{name}_{t}")
            for half in range(2):
                s = bass.ts(half, HALF)
                if t == 0:
                    nc.vector.tensor_tensor(c_tile[:, s], gates[0][:, s],
                                            gates[2][:, s], MULT)
                else:
                    nc.vector.tensor_tensor(c_tile[:, s], t1[:, s],
                                            t2[:, s], ADD)
                # clamp of the PWL output gate (after GpSimd's max lands)
                for q in range(2):
                    sq = bass.ts(2 * half + q, HALF // 2)
                    nc.gpsimd.tensor_scalar(out=o_gate[:, sq],
                                            in0=gates[3][:, sq],
                                            scalar1=0.0, scalar2=1.0,
                                            op0=MAXOP, op1=MINOP)
                nc.scalar.activation(out=tc_t[:, s], in_=c_tile[:, s],
                                     func=TANH)
                for q in range(2):
                    sq = bass.ts(2 * half + q, HALF // 2)
                    nc.vector.tensor_tensor(h_new[:, sq], o_gate[:, sq],
                                            tc_t[:, sq], MULT)
            return h_new, c_tile

        for sb in range(N_SB):
            col0 = sb * SBW

            # ---------------- encoder ----------------
            ys = [None] * T
            h0 = c0 = h1 = c1 = None
            for slot in range(T + 1):
                if slot < T:
                    x_t = xpool.tile([D + 1, SBW], BF16, tag="x", name=f"x_{sb}_{slot}")
                    nc.gpsimd.dma_start(
                        out=x_t, in_=x_ext[slot, :, col0:col0 + SBW])
                    h0, c0 = lstm_step("enc0", slot, x_t, h0, c0,
                                       psA, "gpsA", "A", hbufs=3)
                    ys[slot] = h0
                if slot >= 1:
                    h1, c1 = lstm_step("enc1", slot - 1, ys[slot - 1], h1, c1,
                                       psB, "gpsB", "B", hbufs=2)

            # ---------------- latent ----------------
            z_t = zpool.tile([L + 1, SBW], BF16, tag="z", name=f"z_{sb}")
            for half in range(2):
                gps = psB.tile([H, HALF], F32, tag="gpsB", bufs=2,
                               name=f"lat_{sb}_{half}")
                for cc in range(N_CHUNKS // 2):
                    c = half * (N_CHUNKS // 2) + cc
                    nc.tensor.matmul(gps[:L, bass.ts(cc, CHUNK)], w_lat,
                                     h1[:, bass.ts(c, CHUNK)],
                                     start=True, stop=True)
                nc.scalar.activation(out=z_t[:L, bass.ts(half, HALF)],
                                     in_=gps[:L, :], func=IDENT, bias=b_lat)
            nc.gpsimd.memset(z_t[L:L + 1, :], 1.0)

            # ---------------- decoder ----------------
            d1 = [None] * T
            hd0 = cd0 = hd1 = cd1 = None
            for slot in range(T + 1):
                if slot < T:
                    hd0, cd0 = lstm_step("dec0", slot, z_t, hd0, cd0,
                                         psA, "gpsA", "A", hbufs=3)
                    d1[slot] = hd0
                if slot >= 1:
                    td = slot - 1
                    hd1, cd1 = lstm_step("dec1", td, d1[td], hd1, cd1,
                                         psB, "gpsB", "B", hbufs=2)
                    o_t = opool.tile([H, HALF], F32, tag="o",
                                     name=f"o_{td}")
                    gps = psB.tile([H, HALF], F32, tag="gpsB", bufs=2,
                                   name=f"op_{td}")
                    for half in range(2):
                        row = 64 * half
                        for cc in range(N_CHUNKS // 2):
                            c = half * (N_CHUNKS // 2) + cc
                            nc.tensor.matmul(
                                gps[row:row + 64, bass.ts(cc, CHUNK)], w_out,
                                hd1[:, bass.ts(c, CHUNK)],
                                start=True, stop=True)
                    nc.scalar.activation(out=o_t, in_=gps[:, :],
                                         func=IDENT, bias=b_out)
                    nc.sync.dma_start(
                        out=out_ext[td, :, col0:col0 + HALF],
                        in_=o_t[:D, :])
                    nc.sync.dma_start(
                        out=out_ext[td, :, col0 + HALF:col0 + SBW],
                        in_=o_t[64:64 + D, :])

    nc.finalize()
    return nc


def _prep_inputs(inputs):
    """Host-side: transpose/pack fp32 inputs into per-core device arrays."""
    x = inputs["x"]
    xt = np.ascontiguousarray(np.transpose(x, (1, 2, 0)))   # [T, D, B]
    ones = np.ones((T, 1, B), np.float32)
    xt = np.concatenate([xt, ones], axis=1).astype(ml_dtypes.bfloat16)

    common = {}
    for name in LAYERS:
        Wih = inputs[f"{name}_Wih"]
        Whh = inputs[f"{name}_Whh"]
        bsum = (inputs[f"{name}_bih"] + inputs[f"{name}_bhh"]).astype(np.float32)
        w_in = Wih.T.astype(np.float32)                      # [Din, 4H]
        if not LAYERS[name]["act_bias"]:
            w_in = np.concatenate([w_in, bsum[None, :]], axis=0)
        common[f"{name}_w_in"] = w_in.astype(ml_dtypes.bfloat16)
        common[f"{name}_w_rec"] = Whh.T.astype(ml_dtypes.bfloat16)
        if LAYERS[name]["act_bias"]:
            bias_hg = np.ascontiguousarray(bsum.reshape(4, H).T)  # [H, 4]
            common[f"{name}_bias"] = bias_hg
            common[f"{name}_oline"] = np.ascontiguousarray(
                (PWL_M1 * bias_hg[:, 3] + 0.5).reshape(H, 1)).astype(np.float32)
        else:
            common[f"{name}_oline"] = np.full((H, 1), 0.5, np.float32)
    common["w_lat"] = inputs["W_lat"].T.astype(ml_dtypes.bfloat16)   # [H, L]
    common["b_lat"] = inputs["b_lat"].reshape(L, 1).astype(np.float32)
    w_out_pad = np.zeros((H, 64), np.float32)
    w_out_pad[:, :D] = inputs["W_out"].T
    common["w_out"] = w_out_pad.astype(ml_dtypes.bfloat16)   # [H, 64]
    b_out_pad = np.zeros((128, 1), np.float32)
    b_out_pad[:D, 0] = inputs["b_out"]
    b_out_pad[64:64 + D, 0] = inputs["b_out"]
    common["b_out"] = b_out_pad

    in_maps = []
    for core in range(N_CORES):
        m = dict(common)
        sl = slice(core * B_CORE, (core + 1) * B_CORE)
        m["x"] = np.ascontiguousarray(xt[:, :, sl])
        in_maps.append(m)
    return in_maps


def bench(inputs, reps: int = 8, reuse_nc=None):
    """Time repeated on-device executions (inputs device-resident, outputs
    left on device). Returns (best_seconds, all_times, outputs_of_first_run).
    """
    import jax
    from jax.sharding import Mesh, NamedSharding, PartitionSpec
    from jax.experimental.shard_map import shard_map
    from concourse import bass2jax
    from concourse.bass2jax import _bass_exec_p, partition_id_tensor

    bass2jax.install_neuronx_cc_hook()
    nc = reuse_nc if reuse_nc is not None else _build_kernel()
    in_maps = _prep_inputs(inputs)
    n_cores = N_CORES

    partition_name = nc.partition_id_tensor.name if nc.partition_id_tensor else None
    in_names, out_names, out_avals, zero_outs = [], [], [], []
    for alloc in nc.m.functions[0].allocations:
        if not isinstance(alloc, mybir.MemoryLocationSet):
            continue
        name = alloc.memorylocations[0].name
        if alloc.kind == "ExternalInput":
            if name != partition_name:
                in_names.append(name)
        elif alloc.kind == "ExternalOutput":
            out_names.append(name)
            out_avals.append(
                jax.core.ShapedArray(tuple(alloc.tensor_shape),
                                     mybir.dt.np(alloc.dtype)))
            zero_outs.append(
                np.zeros(tuple(alloc.tensor_shape), mybir.dt.np(alloc.dtype)))
    n_params = len(in_names)
    n_outs = len(out_names)
    all_in_names = in_names + out_names + ([partition_name] if partition_name else [])
    donate = tuple(range(n_params, n_params + n_outs))

    def _body(*args):
        operands = list(args)
        if partition_name is not None:
            operands.append(partition_id_tensor())
        return tuple(_bass_exec_p.bind(
            *operands, out_avals=tuple(out_avals), in_names=tuple(all_in_names),
            out_names=tuple(out_names), lowering_input_output_aliases=(),
            sim_require_finite=True, sim_require_nnan=True, nc=nc))

    devices = jax.devices()[:n_cores]
    mesh = Mesh(np.asarray(devices), ("core",))
    in_specs = (PartitionSpec("core"),) * (n_params + n_outs)
    out_specs = (PartitionSpec("core"),) * n_outs
    sharded = jax.jit(
        shard_map(_body, mesh=mesh, in_specs=in_specs, out_specs=out_specs,
                  check_rep=False),
        donate_argnums=donate, keep_unused=True)

    shard = NamedSharding(mesh, PartitionSpec("core"))
    concat_in = [
        jax.device_put(
            np.concatenate([np.asarray(in_maps[c][nm]) for c in range(n_cores)], 0),
            shard)
        for nm in in_names
    ]
    def fresh_zeros():
        return [jax.device_put(
                    np.zeros((n_cores * z.shape[0], *z.shape[1:]), z.dtype), shard)
                for z in zero_outs]

    # warm-up (compile)
    outs0 = sharded(*concat_in, *fresh_zeros())
    jax.block_until_ready(outs0)

    zero_sets = [fresh_zeros() for _ in range(reps)]
    jax.block_until_ready(zero_sets)
    times = []
    for r in range(reps):
        t0 = time.perf_counter()
        outs = sharded(*concat_in, *zero_sets[r])
        jax.block_until_ready(outs)
        times.append(time.perf_counter() - t0)
    return min(times), times, outs0


def kernel(**inputs) -> np.ndarray:
    global _last_results
    trace = bool(int(os.environ.get("BASS_LSTM_TRACE", "0")))
    nc = _build_kernel(trace)
    in_maps = _prep_inputs(inputs)
    res = run_bass_kernel_spmd(nc, in_maps, core_ids=list(range(N_CORES)),
                               trace=trace)
    _last_results = res
    outs = [res.results[c]["out"] for c in range(N_CORES)]   # [T, D, B_CORE]
    full = np.concatenate(outs, axis=2)                      # [T, D, B]
    return np.ascontiguousarray(np.transpose(full, (2, 0, 1)))  # [B, T, D]

